# revision 3
# baseline (speedup 1.0000x reference)
"""Bass/Trainium2 kernel for NeuralODEBlock (RK4 scan over a 3-layer MLP).

Data-parallel over 8 NeuronCores: h [8192,512] sharded along batch (1024
rows/core), MLP weights replicated. Each core runs the full 10-step RK4 scan
locally; no cross-core communication.

Per-core math (B=1024 batch shard, H=512, H2=1024):
  activations kept transposed: z = x^T [H, B] with H on partitions.
  a1 = W1 @ z  -> tanh(+bias1)   [H2, B]
  a2 = W2 @ t1 -> tanh(+b2)      [H2, B]
  a3 = W3 @ t2                   [H, B]
The b3 bias and the time-embedding W1@(t*wt+bt) are folded into per-eval
bias vectors for the first tanh (computed on host in float64); b3's direct
contribution to the state update is deferred and added once at the end
(h_true = h_stored + s*dt*b3 invariant).
"""

import os
import sys

sys.path.insert(0, "/opt/trn_rl_repo")

import numpy as np

import concourse.bass as bass  # noqa: F401  (registers engine types)
import concourse.mybir as mybir
from concourse import bacc
from concourse.tile import TileContext

H = 512
H2 = 1024
BATCH = 8192
NCORES = 8
BS = BATCH // NCORES  # 1024 batch rows per core
KH = H // 128  # 4
KH2 = H2 // 128  # 8
NT = BS // 512  # 2 moving-dim tiles of 512
P = 128

# compute dtype variant: fp32 | fp32r | bf16  (storage for fp32r is fp32;
# only the matmul operand APs are bitcast to float32r)
VARIANT = os.environ.get("NODE_VARIANT", "bf16")

_f32 = mybir.dt.float32


def _pack_pm(a: np.ndarray) -> np.ndarray:
    """[R, C] with R = r*128  ->  [128, r, C] partition-tiled layout."""
    r = a.shape[0] // P
    return np.ascontiguousarray(a.reshape(r, P, a.shape[1]).transpose(1, 0, 2))


def _build(n_steps: int, variant: str):
    """Build + compile the per-core Bass program. Returns the Bacc object."""
    S = n_steps
    dtc = 1.0 / S  # dt, host-side float64
    if variant == "bf16":
        cdt = mybir.dt.bfloat16
    else:
        cdt = _f32
    mmdt = {"fp32": _f32, "fp32r": mybir.dt.float32r, "bf16": mybir.dt.bfloat16}[
        variant
    ]

    def mm(ap):
        return ap.bitcast(mmdt) if variant == "fp32r" else ap

    nc = bacc.Bacc("TRN2", target_bir_lowering=False, debug=False)
    h_d = nc.dram_tensor("h", [P, KH, BS], _f32, kind="ExternalInput").ap()
    w1_d = nc.dram_tensor("w1t", [P, KH, H2], cdt, kind="ExternalInput").ap()
    w2_d = nc.dram_tensor("w2t", [P, KH2, H2], cdt, kind="ExternalInput").ap()
    w3_d = nc.dram_tensor("w3t", [P, KH2, H], cdt, kind="ExternalInput").ap()
    b1_d = nc.dram_tensor("bias1", [P, 4 * S * 8], _f32, kind="ExternalInput").ap()
    b2_d = nc.dram_tensor("bias2", [P, KH2], _f32, kind="ExternalInput").ap()
    fb_d = nc.dram_tensor("finb", [P, KH], _f32, kind="ExternalInput").ap()
    out_d = nc.dram_tensor("out", [P, KH, BS], _f32, kind="ExternalOutput").ap()

    Tanh = mybir.ActivationFunctionType.Tanh
    Ident = mybir.ActivationFunctionType.Identity
    MUL = mybir.AluOpType.mult
    ADD = mybir.AluOpType.add

    with TileContext(nc) as tc:
        with (
            tc.tile_pool(name="consts", bufs=1) as cp,
            tc.tile_pool(name="state", bufs=1) as sp,
            tc.tile_pool(name="psum", bufs=4, space="PSUM") as pp,
        ):
            w1 = cp.tile([P, KH, H2], cdt, name="w1")
            w2 = cp.tile([P, KH2, H2], cdt, name="w2")
            w3 = cp.tile([P, KH2, H], cdt, name="w3")
            b1t = cp.tile([P, 4 * S * 8], _f32, name="b1t")
            b2t = cp.tile([P, KH2], _f32, name="b2t")
            fbt = cp.tile([P, KH], _f32, name="fbt")
            nc.sync.dma_start(out=w1[:], in_=w1_d)
            nc.sync.dma_start(out=w2[:], in_=w2_d)
            nc.sync.dma_start(out=w3[:], in_=w3_d)
            nc.sync.dma_start(out=b1t[:], in_=b1_d)
            nc.sync.dma_start(out=b2t[:], in_=b2_d)
            nc.sync.dma_start(out=fbt[:], in_=fb_d)

            hh = [sp.tile([P, BS], _f32, name=f"hh{m}", tag=f"hh{m}") for m in range(KH)]
            acc = [sp.tile([P, BS], _f32, name=f"acc{m}", tag=f"acc{m}") for m in range(KH)]
            z = [sp.tile([P, BS], cdt, name=f"z{k}", tag=f"z{k}") for k in range(KH)]
            t1 = [sp.tile([P, BS], cdt, name=f"t1_{k}", tag=f"t1_{k}") for k in range(KH2)]
            t2 = [sp.tile([P, BS], cdt, name=f"t2_{k}", tag=f"t2_{k}") for k in range(KH2)]
            outt = [sp.tile([P, BS], _f32, name=f"o{m}", tag=f"o{m}") for m in range(KH)]

            for m in range(KH):
                nc.sync.dma_start(out=hh[m][:], in_=h_d[:, m, :])
                nc.vector.tensor_copy(out=z[m][:], in_=hh[m][:])

            w_acc = [dtc / 6.0, dtc / 3.0, dtc / 3.0, dtc / 6.0]
            c_next = [dtc / 2.0, dtc / 2.0, dtc, None]

            for s in range(S):
                for i in range(4):
                    e = s * 4 + i
                    # ---- layer 1: a1 = W1 @ z, t1 = tanh(a1 + bias1[e]) ----
                    for m in range(KH2):
                        p1 = pp.tile([P, BS], _f32, name="p1", tag="ps")
                        for n in range(NT):
                            for k in range(KH):
                                nc.tensor.matmul(
                                    p1[:, n * 512 : (n + 1) * 512],
                                    mm(w1[:, k, m * P : (m + 1) * P]),
                                    mm(z[k][:, n * 512 : (n + 1) * 512]),
                                    start=(k == 0),
                                    stop=(k == KH - 1),
                                )
                        nc.scalar.activation(
                            out=t1[m][:],
                            in_=p1[:],
                            func=Tanh,
                            bias=b1t[:, e * 8 + m : e * 8 + m + 1],
                            scale=1.0,
                        )
                    # ---- layer 2: a2 = W2 @ t1, t2 = tanh(a2 + b2) ----
                    for m in range(KH2):
                        p2 = pp.tile([P, BS], _f32, name="p2", tag="ps")
                        for n in range(NT):
                            for k in range(KH2):
                                nc.tensor.matmul(
                                    p2[:, n * 512 : (n + 1) * 512],
                                    mm(w2[:, k, m * P : (m + 1) * P]),
                                    mm(t1[k][:, n * 512 : (n + 1) * 512]),
                                    start=(k == 0),
                                    stop=(k == KH2 - 1),
                                )
                        nc.scalar.activation(
                            out=t2[m][:],
                            in_=p2[:],
                            func=Tanh,
                            bias=b2t[:, m : m + 1],
                            scale=1.0,
                        )
                    # ---- layer 3: a3 = W3 @ t2; RK4 state updates ----
                    for m in range(KH):
                        p3 = pp.tile([P, BS], _f32, name="p3", tag="ps")
                        for n in range(NT):
                            for k in range(KH2):
                                nc.tensor.matmul(
                                    p3[:, n * 512 : (n + 1) * 512],
                                    mm(w3[:, k, m * P : (m + 1) * P]),
                                    mm(t2[k][:, n * 512 : (n + 1) * 512]),
                                    start=(k == 0),
                                    stop=(k == KH2 - 1),
                                )
                        if i < 3:
                            # z_{i+1} = c_{i+1} * a3 + h   (b3 folded into bias1)
                            nc.vector.scalar_tensor_tensor(
                                out=z[m][:], in0=p3[:], scalar=float(c_next[i]),
                                in1=hh[m][:], op0=MUL, op1=ADD,
                            )
                        if i == 0:
                            nc.vector.scalar_tensor_tensor(
                                out=acc[m][:], in0=p3[:], scalar=float(w_acc[0]),
                                in1=hh[m][:], op0=MUL, op1=ADD,
                            )
                        elif i < 3:
                            nc.vector.scalar_tensor_tensor(
                                out=acc[m][:], in0=p3[:], scalar=float(w_acc[i]),
                                in1=acc[m][:], op0=MUL, op1=ADD,
                            )
                        else:
                            nc.vector.scalar_tensor_tensor(
                                out=hh[m][:], in0=p3[:], scalar=float(w_acc[3]),
                                in1=acc[m][:], op0=MUL, op1=ADD,
                            )
                            if s < S - 1:
                                nc.vector.tensor_copy(out=z[m][:], in_=hh[m][:])
                            else:
                                # h_out = h_stored + 1.0 * b3 (deferred bias)
                                nc.scalar.activation(
                                    out=outt[m][:], in_=hh[m][:], func=Ident,
                                    bias=fbt[:, m : m + 1], scale=1.0,
                                )
                                nc.sync.dma_start(out=out_d[:, m, :], in_=outt[m][:])

    nc.compile()
    return nc


def _host_prep(h, W1, b1, W2, b2, W3, b3, Wt, bt, n_steps):
    """Shard + transpose inputs, compute folded bias vectors (float64)."""
    S = int(n_steps)
    dtc = 1.0 / S
    if VARIANT == "bf16":
        import ml_dtypes

        wdt = ml_dtypes.bfloat16
    else:
        wdt = np.float32

    w1t = _pack_pm(np.ascontiguousarray(W1.T)).astype(wdt)  # [128,4,1024]
    w2t = _pack_pm(np.ascontiguousarray(W2.T)).astype(wdt)  # [128,8,1024]
    w3t = _pack_pm(np.ascontiguousarray(W3.T)).astype(wdt)  # [128,8,512]

    W1d = W1.astype(np.float64)
    u = W1d @ Wt[:, 0].astype(np.float64)  # W1 @ wt   [H2]
    v = W1d @ bt.astype(np.float64)  # W1 @ bt   [H2]
    w = W1d @ b3.astype(np.float64)  # W1 @ b3   [H2]
    b1d = b1.astype(np.float64)
    coff = [0.0, dtc / 2.0, dtc / 2.0, dtc]
    bias1 = np.empty((4 * S, H2), np.float64)
    for s in range(S):
        for i in range(4):
            a = s * dtc + coff[i]  # == t_{s,i} and the deferred-b3 coefficient
            bias1[s * 4 + i] = b1d + a * u + v + a * w
    # [4S, H2] -> [128, 4S*8] with column index e*8+m
    bias1_t = (
        bias1.reshape(4 * S, KH2, P).transpose(2, 0, 1).reshape(P, 4 * S * KH2)
    )
    bias1_t = np.ascontiguousarray(bias1_t).astype(np.float32)
    b2t = np.ascontiguousarray(b2.reshape(KH2, P).T).astype(np.float32)
    fbt = np.ascontiguousarray(b3.reshape(KH, P).T).astype(np.float32)

    in_maps = []
    for c in range(NCORES):
        hs = h[c * BS : (c + 1) * BS]  # [1024, 512]
        ht = _pack_pm(np.ascontiguousarray(hs.T.astype(np.float32)))  # [128,4,1024]
        in_maps.append(
            {
                "h": ht,
                "w1t": w1t,
                "w2t": w2t,
                "w3t": w3t,
                "bias1": bias1_t,
                "bias2": b2t,
                "finb": fbt,
            }
        )
    return in_maps


_CACHE = {}


def _get_runner(n_steps: int):
    """Build the program and a cached jitted 8-core executor."""
    key = (n_steps, VARIANT)
    if key in _CACHE:
        return _CACHE[key]

    import jax
    from jax.sharding import Mesh, PartitionSpec, NamedSharding
    from jax.experimental.shard_map import shard_map
    from concourse import bass2jax
    from concourse.bass2jax import _bass_exec_p, install_neuronx_cc_hook

    nc = _build(n_steps, VARIANT)
    install_neuronx_cc_hook()

    partition_name = nc.partition_id_tensor.name if nc.partition_id_tensor else None
    in_names = []
    out_names = []
    out_avals = []
    for alloc in nc.m.functions[0].allocations:
        if not isinstance(alloc, mybir.MemoryLocationSet):
            continue
        name = alloc.memorylocations[0].name
        if alloc.kind == "ExternalInput":
            if name != partition_name:
                in_names.append(name)
        elif alloc.kind == "ExternalOutput":
            import jax.core

            out_names.append(name)
            shape = tuple(alloc.tensor_shape)
            dtype = mybir.dt.np(alloc.dtype)
            out_avals.append(jax.core.ShapedArray(shape, dtype))
    n_params = len(in_names)
    all_names = in_names + out_names
    if partition_name is not None:
        all_names = all_names + [partition_name]

    def _body(*args):
        operands = list(args)
        if partition_name is not None:
            operands.append(bass2jax.partition_id_tensor())
        outs = _bass_exec_p.bind(
            *operands,
            out_avals=tuple(out_avals),
            in_names=tuple(all_names),
            out_names=tuple(out_names),
            lowering_input_output_aliases=(),
            sim_require_finite=True,
            sim_require_nnan=True,
            nc=nc,
        )
        return tuple(outs)

    devices = jax.devices()[:NCORES]
    mesh = Mesh(np.asarray(devices), ("core",))
    in_specs = (PartitionSpec("core"),) * (n_params + len(out_names))
    out_specs = (PartitionSpec("core"),) * len(out_names)
    sharded = jax.jit(
        shard_map(
            _body, mesh=mesh, in_specs=in_specs, out_specs=out_specs, check_rep=False
        ),
        donate_argnums=tuple(range(n_params, n_params + len(out_names))),
        keep_unused=True,
    )
    runner = {
        "nc": nc,
        "sharded": sharded,
        "in_names": in_names,
        "out_names": out_names,
        "out_avals": out_avals,
        "mesh": mesh,
        "n_params": n_params,
    }
    _CACHE[key] = runner
    return runner


def _run_in_maps(runner, in_maps):
    """Execute; returns list of per-core output dicts."""
    import jax

    n_params = runner["n_params"]
    in_names = runner["in_names"]
    out_avals = runner["out_avals"]
    concat_in = [
        np.concatenate([in_maps[c][nm] for c in range(NCORES)], axis=0)
        for nm in in_names
    ]
    concat_zeros = [
        np.zeros((NCORES * a.shape[0], *a.shape[1:]), a.dtype) for a in out_avals
    ]
    out_arrs = runner["sharded"](*concat_in, *concat_zeros)
    outs = []
    for c in range(NCORES):
        outs.append(
            {
                nm: np.asarray(out_arrs[i]).reshape(NCORES, *out_avals[i].shape)[c]
                for i, nm in enumerate(runner["out_names"])
            }
        )
    return outs


def kernel(h, W1, b1, W2, b2, W3, b3, Wt, bt, n_steps):
    h = np.asarray(h)
    S = int(np.asarray(n_steps))
    runner = _get_runner(S)
    in_maps = _host_prep(h, np.asarray(W1), np.asarray(b1), np.asarray(W2),
                         np.asarray(b2), np.asarray(W3), np.asarray(b3),
                         np.asarray(Wt), np.asarray(bt), S)
    outs = _run_in_maps(runner, in_maps)
    shards = []
    for c in range(NCORES):
        o = outs[c]["out"]  # [128, KH, BS]
        shards.append(np.ascontiguousarray(o.transpose(1, 0, 2).reshape(H, BS).T))
    return np.concatenate(shards, axis=0).astype(np.float32)


# revision 7
# speedup vs baseline: 1.7489x; 1.7489x over previous
"""Bass/Trainium2 kernel for NeuralODEBlock (RK4 scan over a 3-layer MLP).

Data-parallel over 8 NeuronCores: h [8192,512] sharded along batch (1024
rows/core), MLP weights replicated. Each core runs the full 10-step RK4 scan
locally; no cross-core communication.

Per-core math (B=1024 batch shard, H=512, H2=1024):
  activations kept transposed: z = x^T [H, B] with H on partitions.
  a1 = W1 @ z  -> tanh(+bias1)   [H2, B]
  a2 = W2 @ t1 -> tanh(+b2)      [H2, B]
  a3 = W3 @ t2                   [H, B]
The b3 bias and the time-embedding W1@(t*wt+bt) are folded into per-eval
bias vectors for the first tanh (computed on host in float64); b3's direct
contribution to the state update is deferred and added once at the end
(h_true = h_stored + s*dt*b3 invariant).
"""

import os
import sys

sys.path.insert(0, "/opt/trn_rl_repo")

import numpy as np

import concourse.bass as bass  # noqa: F401  (registers engine types)
import concourse.mybir as mybir
from concourse import bacc
from concourse.tile import TileContext

H = 512
H2 = 1024
BATCH = 8192
NCORES = 8
BS = BATCH // NCORES  # 1024 batch rows per core
KH = H // 128  # 4
KH2 = H2 // 128  # 8
NT = BS // 512  # 2 moving-dim tiles of 512
P = 128

# compute dtype variant: fp32 | fp32r | bf16  (storage for fp32r is fp32;
# only the matmul operand APs are bitcast to float32r)
VARIANT = os.environ.get("NODE_VARIANT", "bf16")

_f32 = mybir.dt.float32


def _pack_pm(a: np.ndarray) -> np.ndarray:
    """[R, C] with R = r*128  ->  [128, r, C] partition-tiled layout."""
    r = a.shape[0] // P
    return np.ascontiguousarray(a.reshape(r, P, a.shape[1]).transpose(1, 0, 2))


def _build(n_steps: int, variant: str):
    """Build + compile the per-core Bass program. Returns the Bacc object."""
    S = n_steps
    dtc = 1.0 / S  # dt, host-side float64
    if variant == "bf16":
        cdt = mybir.dt.bfloat16
    else:
        cdt = _f32
    mmdt = {"fp32": _f32, "fp32r": mybir.dt.float32r, "bf16": mybir.dt.bfloat16}[
        variant
    ]

    def mm(ap):
        return ap.bitcast(mmdt) if variant == "fp32r" else ap

    # matmul moving-operand free dim: 512 is the ISA max on this target
    # (s3d3_mm_num_elements check rejects 1024 even for bf16)
    NF = 512
    NNT = BS // NF

    nc = bacc.Bacc("TRN2", target_bir_lowering=False, debug=False)
    h_d = nc.dram_tensor("h", [P, KH, BS], _f32, kind="ExternalInput").ap()
    w1_d = nc.dram_tensor("w1t", [P, KH, H2], cdt, kind="ExternalInput").ap()
    w2_d = nc.dram_tensor("w2t", [P, KH2, H2], cdt, kind="ExternalInput").ap()
    w3_d = nc.dram_tensor("w3t", [P, KH2, H], cdt, kind="ExternalInput").ap()
    b1_d = nc.dram_tensor("bias1", [P, 4 * S * 8], _f32, kind="ExternalInput").ap()
    b2_d = nc.dram_tensor("bias2", [P, KH2], _f32, kind="ExternalInput").ap()
    fb_d = nc.dram_tensor("finb", [P, KH], _f32, kind="ExternalInput").ap()
    out_d = nc.dram_tensor("out", [P, KH, BS], _f32, kind="ExternalOutput").ap()

    Tanh = mybir.ActivationFunctionType.Tanh
    Ident = mybir.ActivationFunctionType.Identity
    MUL = mybir.AluOpType.mult
    ADD = mybir.AluOpType.add

    with TileContext(nc) as tc:
        with (
            tc.tile_pool(name="consts", bufs=1) as cp,
            tc.tile_pool(name="state", bufs=1) as sp,
            tc.tile_pool(name="psum", bufs=4, space="PSUM") as pp,
        ):
            w1 = cp.tile([P, KH, H2], cdt, name="w1")
            w2 = cp.tile([P, KH2, H2], cdt, name="w2")
            w3 = cp.tile([P, KH2, H], cdt, name="w3")
            b1t = cp.tile([P, 4 * S * 8], _f32, name="b1t")
            b2t = cp.tile([P, KH2], _f32, name="b2t")
            fbt = cp.tile([P, KH], _f32, name="fbt")
            hh = [sp.tile([P, BS], _f32, name=f"hh{m}", tag=f"hh{m}") for m in range(KH)]
            acc = [sp.tile([P, BS], _f32, name=f"acc{m}", tag=f"acc{m}") for m in range(KH)]
            z = [sp.tile([P, BS], cdt, name=f"z{k}", tag=f"z{k}") for k in range(KH)]
            t1 = [sp.tile([P, BS], cdt, name=f"t1_{k}", tag=f"t1_{k}") for k in range(KH2)]
            t2 = [sp.tile([P, BS], cdt, name=f"t2_{k}", tag=f"t2_{k}") for k in range(KH2)]
            outt = [sp.tile([P, BS], _f32, name=f"o{m}", tag=f"o{m}") for m in range(KH)]

            # startup order matters: the first matmuls need h (via z) and w1
            # only; w2/w3 can stream in behind layer-1 compute.
            for m in range(KH):
                nc.sync.dma_start(out=hh[m][:], in_=h_d[:, m, :])
                nc.vector.tensor_copy(out=z[m][:], in_=hh[m][:])
            nc.sync.dma_start(out=w1[:], in_=w1_d)
            nc.sync.dma_start(out=b1t[:], in_=b1_d)
            nc.sync.dma_start(out=w2[:], in_=w2_d)
            nc.sync.dma_start(out=b2t[:], in_=b2_d)
            nc.sync.dma_start(out=w3[:], in_=w3_d)
            nc.sync.dma_start(out=fbt[:], in_=fb_d)

            w_acc = [dtc / 6.0, dtc / 3.0, dtc / 3.0, dtc / 6.0]
            c_next = [dtc / 2.0, dtc / 2.0, dtc, None]

            for s in range(S):
                for i in range(4):
                    e = s * 4 + i
                    # ---- layer 1: a1 = W1 @ z, t1 = tanh(a1 + bias1[e]) ----
                    for m in range(KH2):
                        p1 = pp.tile([P, BS], _f32, name="p1", tag="ps")
                        for n in range(NNT):
                            for k in range(KH):
                                nc.tensor.matmul(
                                    p1[:, n * NF : (n + 1) * NF],
                                    mm(w1[:, k, m * P : (m + 1) * P]),
                                    mm(z[k][:, n * NF : (n + 1) * NF]),
                                    start=(k == 0),
                                    stop=(k == KH - 1),
                                )
                        nc.scalar.activation(
                            out=t1[m][:],
                            in_=p1[:],
                            func=Tanh,
                            bias=b1t[:, e * 8 + m : e * 8 + m + 1],
                            scale=1.0,
                        )
                    # ---- layer 2: a2 = W2 @ t1, t2 = tanh(a2 + b2) ----
                    for m in range(KH2):
                        p2 = pp.tile([P, BS], _f32, name="p2", tag="ps")
                        for n in range(NNT):
                            for k in range(KH2):
                                nc.tensor.matmul(
                                    p2[:, n * NF : (n + 1) * NF],
                                    mm(w2[:, k, m * P : (m + 1) * P]),
                                    mm(t1[k][:, n * NF : (n + 1) * NF]),
                                    start=(k == 0),
                                    stop=(k == KH2 - 1),
                                )
                        nc.scalar.activation(
                            out=t2[m][:],
                            in_=p2[:],
                            func=Tanh,
                            bias=b2t[:, m : m + 1],
                            scale=1.0,
                        )
                    # ---- layer 3: a3 = W3 @ t2; RK4 state updates ----
                    for m in range(KH):
                        p3 = pp.tile([P, BS], _f32, name="p3", tag="ps")
                        for n in range(NNT):
                            for k in range(KH2):
                                nc.tensor.matmul(
                                    p3[:, n * NF : (n + 1) * NF],
                                    mm(w3[:, k, m * P : (m + 1) * P]),
                                    mm(t2[k][:, n * NF : (n + 1) * NF]),
                                    start=(k == 0),
                                    stop=(k == KH2 - 1),
                                )
                        if i < 3:
                            # z_{i+1} = c_{i+1} * a3 + h   (b3 folded into bias1)
                            nc.vector.scalar_tensor_tensor(
                                out=z[m][:], in0=p3[:], scalar=float(c_next[i]),
                                in1=hh[m][:], op0=MUL, op1=ADD,
                            )
                        if i == 0:
                            nc.vector.scalar_tensor_tensor(
                                out=acc[m][:], in0=p3[:], scalar=float(w_acc[0]),
                                in1=hh[m][:], op0=MUL, op1=ADD,
                            )
                        elif i < 3:
                            nc.vector.scalar_tensor_tensor(
                                out=acc[m][:], in0=p3[:], scalar=float(w_acc[i]),
                                in1=acc[m][:], op0=MUL, op1=ADD,
                            )
                        else:
                            nc.vector.scalar_tensor_tensor(
                                out=hh[m][:], in0=p3[:], scalar=float(w_acc[3]),
                                in1=acc[m][:], op0=MUL, op1=ADD,
                            )
                            if s < S - 1:
                                nc.vector.tensor_copy(out=z[m][:], in_=hh[m][:])
                            else:
                                # h_out = h_stored + 1.0 * b3 (deferred bias)
                                nc.scalar.activation(
                                    out=outt[m][:], in_=hh[m][:], func=Ident,
                                    bias=fbt[:, m : m + 1], scale=1.0,
                                )
                                nc.sync.dma_start(out=out_d[:, m, :], in_=outt[m][:])

    nc.compile()
    return nc


def _host_prep(h, W1, b1, W2, b2, W3, b3, Wt, bt, n_steps):
    """Shard + transpose inputs, compute folded bias vectors (float64)."""
    S = int(n_steps)
    dtc = 1.0 / S
    if VARIANT == "bf16":
        import ml_dtypes

        wdt = ml_dtypes.bfloat16
    else:
        wdt = np.float32

    w1t = _pack_pm(np.ascontiguousarray(W1.T)).astype(wdt)  # [128,4,1024]
    w2t = _pack_pm(np.ascontiguousarray(W2.T)).astype(wdt)  # [128,8,1024]
    w3t = _pack_pm(np.ascontiguousarray(W3.T)).astype(wdt)  # [128,8,512]

    W1d = W1.astype(np.float64)
    u = W1d @ Wt[:, 0].astype(np.float64)  # W1 @ wt   [H2]
    v = W1d @ bt.astype(np.float64)  # W1 @ bt   [H2]
    w = W1d @ b3.astype(np.float64)  # W1 @ b3   [H2]
    b1d = b1.astype(np.float64)
    coff = [0.0, dtc / 2.0, dtc / 2.0, dtc]
    bias1 = np.empty((4 * S, H2), np.float64)
    for s in range(S):
        for i in range(4):
            a = s * dtc + coff[i]  # == t_{s,i} and the deferred-b3 coefficient
            bias1[s * 4 + i] = b1d + a * u + v + a * w
    # [4S, H2] -> [128, 4S*8] with column index e*8+m
    bias1_t = (
        bias1.reshape(4 * S, KH2, P).transpose(2, 0, 1).reshape(P, 4 * S * KH2)
    )
    bias1_t = np.ascontiguousarray(bias1_t).astype(np.float32)
    b2t = np.ascontiguousarray(b2.reshape(KH2, P).T).astype(np.float32)
    fbt = np.ascontiguousarray(b3.reshape(KH, P).T).astype(np.float32)

    in_maps = []
    for c in range(NCORES):
        hs = h[c * BS : (c + 1) * BS]  # [1024, 512]
        ht = _pack_pm(np.ascontiguousarray(hs.T.astype(np.float32)))  # [128,4,1024]
        in_maps.append(
            {
                "h": ht,
                "w1t": w1t,
                "w2t": w2t,
                "w3t": w3t,
                "bias1": bias1_t,
                "bias2": b2t,
                "finb": fbt,
            }
        )
    return in_maps


_CACHE = {}


def _get_runner(n_steps: int):
    """Build the program and a cached jitted 8-core executor."""
    key = (n_steps, VARIANT)
    if key in _CACHE:
        return _CACHE[key]

    import jax
    from jax.sharding import Mesh, PartitionSpec, NamedSharding
    from jax.experimental.shard_map import shard_map
    from concourse import bass2jax
    from concourse.bass2jax import _bass_exec_p, install_neuronx_cc_hook

    nc = _build(n_steps, VARIANT)
    install_neuronx_cc_hook()

    partition_name = nc.partition_id_tensor.name if nc.partition_id_tensor else None
    in_names = []
    out_names = []
    out_avals = []
    for alloc in nc.m.functions[0].allocations:
        if not isinstance(alloc, mybir.MemoryLocationSet):
            continue
        name = alloc.memorylocations[0].name
        if alloc.kind == "ExternalInput":
            if name != partition_name:
                in_names.append(name)
        elif alloc.kind == "ExternalOutput":
            import jax.core

            out_names.append(name)
            shape = tuple(alloc.tensor_shape)
            dtype = mybir.dt.np(alloc.dtype)
            out_avals.append(jax.core.ShapedArray(shape, dtype))
    n_params = len(in_names)
    all_names = in_names + out_names
    if partition_name is not None:
        all_names = all_names + [partition_name]

    def _body(*args):
        operands = list(args)
        if partition_name is not None:
            operands.append(bass2jax.partition_id_tensor())
        outs = _bass_exec_p.bind(
            *operands,
            out_avals=tuple(out_avals),
            in_names=tuple(all_names),
            out_names=tuple(out_names),
            lowering_input_output_aliases=(),
            sim_require_finite=True,
            sim_require_nnan=True,
            nc=nc,
        )
        return tuple(outs)

    devices = jax.devices()[:NCORES]
    mesh = Mesh(np.asarray(devices), ("core",))
    in_specs = (PartitionSpec("core"),) * (n_params + len(out_names))
    out_specs = (PartitionSpec("core"),) * len(out_names)
    sharded = jax.jit(
        shard_map(
            _body, mesh=mesh, in_specs=in_specs, out_specs=out_specs, check_rep=False
        ),
        donate_argnums=tuple(range(n_params, n_params + len(out_names))),
        keep_unused=True,
    )
    runner = {
        "nc": nc,
        "sharded": sharded,
        "in_names": in_names,
        "out_names": out_names,
        "out_avals": out_avals,
        "mesh": mesh,
        "n_params": n_params,
    }
    _CACHE[key] = runner
    return runner


def _run_in_maps(runner, in_maps):
    """Execute; returns list of per-core output dicts."""
    import jax

    n_params = runner["n_params"]
    in_names = runner["in_names"]
    out_avals = runner["out_avals"]
    concat_in = [
        np.concatenate([in_maps[c][nm] for c in range(NCORES)], axis=0)
        for nm in in_names
    ]
    concat_zeros = [
        np.zeros((NCORES * a.shape[0], *a.shape[1:]), a.dtype) for a in out_avals
    ]
    out_arrs = runner["sharded"](*concat_in, *concat_zeros)
    outs = []
    for c in range(NCORES):
        outs.append(
            {
                nm: np.asarray(out_arrs[i]).reshape(NCORES, *out_avals[i].shape)[c]
                for i, nm in enumerate(runner["out_names"])
            }
        )
    return outs


def kernel(h, W1, b1, W2, b2, W3, b3, Wt, bt, n_steps):
    h = np.asarray(h)
    S = int(np.asarray(n_steps))
    runner = _get_runner(S)
    in_maps = _host_prep(h, np.asarray(W1), np.asarray(b1), np.asarray(W2),
                         np.asarray(b2), np.asarray(W3), np.asarray(b3),
                         np.asarray(Wt), np.asarray(bt), S)
    outs = _run_in_maps(runner, in_maps)
    shards = []
    for c in range(NCORES):
        o = outs[c]["out"]  # [128, KH, BS]
        shards.append(np.ascontiguousarray(o.transpose(1, 0, 2).reshape(H, BS).T))
    return np.concatenate(shards, axis=0).astype(np.float32)
